# revision 38
# baseline (speedup 1.0000x reference)
"""Trainium2 Bass kernel for nn_Attention_87668872446719.

Patch-attention module: v = Conv3x3(x); xe = PatchEmbed(x); q,k = Linear(xe);
attn = softmax(q k^T / sqrt(hd)); out = Fold(attn @ Unfold(v)); out = Conv1x1(out).

Identity used (validated numerically): the unfold/attn/fold pipeline equals,
per channel c with head h = c // 32:
    folded[c, patch n, off] = sum_m attn[h, n, m] * v[c, patch m, off]

Sharding (8 cores, no collectives): core = (image b in 0..3, half s in 0..1).
s splits every 16x16 patch into its top/bottom 8 rows (off = ki*16+kj with
ki in [8s, 8s+8)), so the 1x1 proj stays pixel-local per core and each core
writes disjoint output rows.

Per core on device (all matmuls bf16, f32 PSUM accumulation):
  1. v conv first (warms the PE), TRANSPOSED output: lhsT = im2col slice
     [27, m-chunk] (pixel cols ordered o-major), rhs = wvT[27, 256] ->
     psum[m, 256 c] per o -> evict into VT[mc] = [m, (o, c)] bf16.
     V never leaves SBUF and needs no partition shuffle.
  2. xeT[256,196] = patch embed; qT/kT[32,196] per head (q pre-scaled)
  3. S[n,m] per head -> softmax; 1/rowsum folded into the bf16 cast of A;
     A transposed to AT[m, n] via PE (chunks of 98)
  4. stage E (F^T form): for each c: psum[off(128), n(196)] accumulated
     over m-chunks with lhsT = VT[:, c::256] (o-strided), rhs = AT
     -> fsb[off, (c, n)] -> fdram[off, c, n] (bf16, 12.5KB contiguous
     writes per off-row; the o<->c scatter cost is paid on the read side
     where it overlaps stage E + proj)
  5. proj: out[oc, (off n)] = projw @ F read back as [c, (32 off, n)]
     tiles; bf16 output (host upcasts to f32)
"""
from contextlib import ExitStack

import numpy as np
import ml_dtypes

import concourse.bass as bass
import concourse.tile as tile
from concourse import bacc, mybir
from concourse.bass_utils import run_bass_kernel_spmd

B, CIN, H, W = 4, 3, 224, 224
P = 16
DIM = 256
HEADS = 8
Hp = Wp = 14
N = Hp * Wp            # 196 patches
HD = DIM // HEADS      # 32
KI = 8                 # patch rows per core
OFF = KI * P           # 128 within-patch pixels per core
NPIX = N * OFF         # 25088 pixels per core
MCH = 98               # m-chunk (2 chunks of 98)
NCH = 98               # n-chunk for softmax/transposes
Q = 49                 # conv m-block (4 blocks of 49 m per cc)
BF = mybir.dt.bfloat16
F32 = mybir.dt.float32
AFT = mybir.ActivationFunctionType
AX = mybir.AxisListType.X

_CACHE = {}


def _build():
    nc = bacc.Bacc("TRN2", target_bir_lowering=False, debug=False)

    xcol_d = nc.declare_dram_parameter("xcol", [64, 12544], BF, isOutput=False)
    patches_d = nc.declare_dram_parameter("patches", [128, 6, N], BF, isOutput=False)
    pwT_d = nc.declare_dram_parameter("pwT", [128, 6, DIM], BF, isOutput=False)
    qkwT_d = nc.declare_dram_parameter("qkwT", [128, 2, 2 * DIM], BF, isOutput=False)
    wvT_d = nc.declare_dram_parameter("wvT", [64, DIM], BF, isOutput=False)
    projwT_d = nc.declare_dram_parameter("projwT", [128, 2, DIM], BF, isOutput=False)
    pbias_d = nc.declare_dram_parameter("pbias", [128, 2], F32, isOutput=False)
    obias_d = nc.declare_dram_parameter("obias", [128, 2], F32, isOutput=False)
    ident_d = nc.declare_dram_parameter("ident", [NCH, NCH], BF, isOutput=False)
    out_d = nc.declare_dram_parameter("out", [DIM, NPIX], BF, isOutput=True)

    # F split: heads 0-3 c-major (scattered writes early, contiguous
    # reads); heads 4-7 off-major (contiguous writes, scattered reads
    # pipelined under proj).
    fdramA = nc.dram_tensor("fdramA", [128, OFF, N], BF)     # [c, off, n]
    fdramB = nc.dram_tensor("fdramB", [OFF, 128, N], BF)     # [off, c, n]

    with tile.TileContext(nc) as tc, ExitStack() as ctx:
        const = ctx.enter_context(tc.tile_pool(name="const", bufs=1))
        stat = ctx.enter_context(tc.tile_pool(name="stat", bufs=4))
        sb = ctx.enter_context(tc.tile_pool(name="sb", bufs=2))
        atp = ctx.enter_context(tc.tile_pool(name="atp", bufs=1))
        pP = ctx.enter_context(tc.tile_pool(name="pP", bufs=2, space="PSUM"))
        pA = ctx.enter_context(tc.tile_pool(name="pA", bufs=3, space="PSUM"))
        vctx = ctx.enter_context(ExitStack())
        vtp = vctx.enter_context(tc.tile_pool(name="vtp", bufs=1))

        # ---- constants (spread across issue queues; xcol gates conv and
        # is issued first, on its own queue) ----
        qrot = [nc.scalar, nc.sync]

        def cload(shape, dt, dram, tag, qi=[0]):
            t = const.tile(shape, dt, tag=tag, name=tag)
            q = qrot[qi[0] % 2]
            qi[0] += 1
            q.dma_start(t[:], dram[:])
            return t

        with tc.high_priority():
            wvT_t = cload([64, DIM], BF, wvT_d, "c_wvT")
        patches_t = cload([128, 6, N], BF, patches_d, "c_patches")
        pwT_t = cload([128, 6, DIM], BF, pwT_d, "c_pwT")
        qkwT_t = cload([128, 2, 2 * DIM], BF, qkwT_d, "c_qkwT")
        projwT_t = cload([128, 2, DIM], BF, projwT_d, "c_projwT")
        pbias_t = cload([128, 2], F32, pbias_d, "c_pbias")
        obias_t = cload([128, 2], F32, obias_d, "c_obias")
        ident_t = cload([NCH, NCH], BF, ident_d, "c_ident")

        # VT[mc]: [msz, (128 off, 256 c)] bf16, partition = m
        MSZ = (128, 68)
        VT = [vtp.tile([MSZ[mc], OFF * DIM], BF, tag="vt%d" % mc,
                       name="vt%d" % mc) for mc in range(2)]

        ev_flip = [0]

        def evict(dst, src, scale=None, bias=None):
            """PSUM -> SBUF eviction alternating DVE / ACT."""
            e = ev_flip[0] = 1 - ev_flip[0]
            if scale is not None:
                if e:
                    nc.vector.tensor_scalar_mul(dst, src, scale)
                else:
                    nc.scalar.activation(dst, src, AFT.Copy, scale=scale)
            elif bias is not None:
                if e:
                    nc.vector.tensor_scalar_add(dst, src, bias)
                else:
                    nc.scalar.activation(dst, src, AFT.Identity, bias=bias)
            else:
                if e:
                    nc.vector.tensor_copy(dst, src)
                else:
                    nc.scalar.copy(dst, src)

        # ---- stage D first (warms PE early): v conv, transposed out ----
        # xcol cols ordered (o-major, m-minor): col = o_local*196 + m,
        # group gr = o // 64 at partition base 32*gr.
        with tc.tile_pool(name="px", bufs=1) as px:
            xcol_t = px.tile([64, 64 * N], BF, tag="xcol", name="xcol")
            with tc.high_priority():
                for i4 in range(4):
                    q = nc.sync if i4 % 2 == 0 else nc.scalar
                    q.dma_start(xcol_t[:, i4 * 16 * N:(i4 + 1) * 16 * N],
                                xcol_d[:, i4 * 16 * N:(i4 + 1) * 16 * N])
            for o4 in range(32):      # 4 off per psum tile
                for mc in range(2):
                    msz = MSZ[mc]
                    ps = pA.tile([128, 1024], F32, tag="mm", name="psc")
                    for i in range(4):
                        o = o4 * 4 + i
                        gr, ol = divmod(o, 64)
                        nc.tensor.matmul(
                            ps[:msz, i * 256:(i + 1) * 256],
                            xcol_t[32 * gr:32 * gr + 27,
                                   ol * N + mc * 128:ol * N + mc * 128 + msz],
                            wvT_t[32 * gr:32 * gr + 27, :],
                            start=True, stop=True)
                    evict(VT[mc][:, o4 * 1024:(o4 + 1) * 1024], ps[:msz])

        # ---- stage A: xeT[c, n] = patch embed (transposed) ----
        xeT = []
        for cc in range(2):
            ps = pP.tile([128, N], F32, tag="sm", name="pse")
            for kc in range(6):
                nc.tensor.matmul(
                    ps[:], pwT_t[:, kc, cc * 128:(cc + 1) * 128],
                    patches_t[:, kc, :], start=(kc == 0), stop=(kc == 5))
            xt = sb.tile([128, N], BF, tag="xeT%d" % cc, name="xeT")
            nc.vector.tensor_scalar_add(xt[:], ps[:], pbias_t[:, cc:cc + 1])
            xeT.append(xt)

        # ---- stage B/C: per-head q/k, scores, softmax, AT ----
        AT = []     # AT[h][mc] : [98, 196] bf16 (A^T, normalized)
        for h in range(HEADS):
            qT = sb.tile([HD, N], BF, tag="qT", name="qT")
            kT = sb.tile([HD, N], BF, tag="kT", name="kT")
            for dst, joff in ((qT, h * HD), (kT, DIM + h * HD)):
                ps = pP.tile([HD, N], F32, tag="sm", name="psq")
                for cc in range(2):
                    nc.tensor.matmul(
                        ps[:], qkwT_t[:, cc, joff:joff + HD], xeT[cc][:],
                        start=(cc == 0), stop=(cc == 1))
                nc.scalar.copy(dst[:], ps[:])

            Ah = []
            for nci in range(2):
                nb = nci * NCH
                ps = pP.tile([NCH, N], F32, tag="sm", name="pss")
                nc.tensor.matmul(ps[:], qT[:, nb:nb + NCH], kT[:],
                                 start=True, stop=True)
                mx = stat.tile([NCH, 1], F32, tag="mx", name="mx")
                nc.vector.reduce_max(mx[:], ps[:], axis=AX, negate=True)
                ex = sb.tile([NCH, N], BF, tag="ex", name="ex")
                nc.scalar.activation(ex[:], ps[:], AFT.Exp, bias=mx[:])
                sm = stat.tile([NCH, 1], F32, tag="smm", name="smm")
                nc.vector.reduce_sum(sm[:], ex[:], axis=AX)
                rc = stat.tile([NCH, 1], F32, tag="rc", name="rc")
                nc.vector.reciprocal(rc[:], sm[:])
                ab = sb.tile([NCH, N], BF, tag="ab", name="ab")
                nc.vector.tensor_scalar_mul(ab[:], ex[:], rc[:])
                Ah.append(ab)

            ATh = []
            for mc in range(2):
                msz = MSZ[mc]
                at = atp.tile([msz, N], BF, tag="at%d_%d" % (mc, h), name="at")
                mb = mc * 128
                for nci in range(2):
                    nb = nci * NCH
                    pt = pP.tile([msz, NCH], BF, tag="sm", name="pst")
                    nc.tensor.transpose(pt[:], Ah[nci][:, mb:mb + msz],
                                        ident_t[:])
                    evict(at[:, nb:nb + NCH], pt[:])
                ATh.append(at)
            AT.append(ATh)

        # ---- stage E (F^T form): psum[off, n] per c, evict to fsb ----
        VTv = [VT[mc].rearrange("m (o c) -> m o c", c=DIM) for mc in range(2)]
        with tc.tile_pool(name="fsp", bufs=2) as fsp:
            for h in range(HEADS):
                fsb = fsp.tile([128, 32 * N], BF, tag="fsb", name="fsb")
                for jj in range(8):   # groups of 4 c
                    ps = pA.tile([128, 1024], F32, tag="mm", name="psf")
                    for j2 in range(4):
                        cg = h * 32 + jj * 4 + j2
                        o0 = (j2 // 2) * 512 + (j2 % 2) * N
                        for mc in range(2):
                            nc.tensor.matmul(
                                ps[:, o0:o0 + N],
                                VTv[mc][:, :, cg],
                                AT[h][mc][:],
                                start=(mc == 0), stop=(mc == 1))
                    src = ps[:].rearrange("p (b x) -> p b x", b=2)[:, :, :2 * N]
                    dst = fsb[:, jj * 4 * N:(jj + 1) * 4 * N].rearrange(
                        "p (b x) -> p b x", b=2)
                    evict(dst, src)
                # fsb[off, (32c, 196n)] -> fdram
                if h < 4:
                    fd = fdramA[h * HD:(h + 1) * HD, :, :].rearrange(
                        "c o n -> o c n")
                else:
                    fd = fdramB[:, (h - 4) * HD:(h - 3) * HD, :]
                nc.sync.dma_start(
                    fd, fsb[:].rearrange("o (c n) -> o c n", n=N))

        vctx.close()   # free VT before proj tiles allocate

        # ---- stage F: proj from fdram, bf16 out ----
        # fr tiles [c(128), (32 off, 196 n)] per (ogrp, cc); out cols
        # remain (off, n) order.
        GW = 32 * N  # 6272
        with tc.tile_pool(name="pfr", bufs=2) as pfr, \
             tc.tile_pool(name="posb", bufs=2) as posb:
            for og in range(4):
                frs = []
                for cc in range(2):
                    fr = pfr.tile([128, GW], BF, tag="fr%d" % cc, name="fr")
                    if cc == 0:
                        src = fdramA[:, og * 32:(og + 1) * 32, :]
                    else:
                        src = fdramB[og * 32:(og + 1) * 32, :, :].rearrange(
                            "o c n -> c o n")
                    nc.sync.dma_start(
                        fr[:].rearrange("c (o n) -> c o n", n=N), src)
                    frs.append(fr)
                for occ in range(2):
                    ot = posb.tile([128, GW], BF, tag="osb", name="osb")
                    for t6 in range(7):
                        w = 1024 if t6 < 6 else 128
                        ps = pA.tile([128, 1024], F32, tag="mm", name="psp")
                        for half in range((w + 511) // 512):
                            b0 = t6 * 1024 + half * 512
                            bw = min(512, w - half * 512)
                            for cc in range(2):
                                nc.tensor.matmul(
                                    ps[:, half * 512:half * 512 + bw],
                                    projwT_t[:, cc, occ * 128:(occ + 1) * 128],
                                    frs[cc][:, b0:b0 + bw],
                                    start=(cc == 0), stop=(cc == 1))
                        evict(ot[:, t6 * 1024:t6 * 1024 + w], ps[:, :w],
                              bias=obias_t[:, occ:occ + 1])
                    nc.sync.dma_start(
                        out_d[occ * 128:(occ + 1) * 128,
                              og * GW:(og + 1) * GW], ot[:])

    nc.compile()
    return nc


def _host_prep(inputs):
    """Returns per-core in_maps."""
    x = np.asarray(inputs["x"], np.float32)
    patch_w = np.asarray(inputs["patch_w"], np.float32)
    patch_b = np.asarray(inputs["patch_b"], np.float32)
    qk_w = np.asarray(inputs["qk_w"], np.float32)
    v_w = np.asarray(inputs["v_w"], np.float32)
    v_b = np.asarray(inputs["v_b"], np.float32)
    proj_w = np.asarray(inputs["proj_w"], np.float32).reshape(DIM, DIM)
    proj_b = np.asarray(inputs["proj_b"], np.float32)

    bf = ml_dtypes.bfloat16
    pw = patch_w.reshape(DIM, CIN * P * P)                     # [256, 768]
    pwT = pw.T.reshape(6, 128, DIM).transpose(1, 0, 2)         # [128, 6, 256]
    qkw = qk_w.copy()
    qkw[:DIM] *= HD ** -0.5                                    # fold attn scale
    qkwT = qkw.T.reshape(2, 128, 2 * DIM).transpose(1, 0, 2)   # [128, 2, 512]
    wvT = v_w.reshape(DIM, 27).T                               # [27, 256]
    wvT4 = np.zeros((64, DIM), np.float32)                     # 32-aligned x2
    wvT4[0:27] = wvT
    wvT4[32:59] = wvT
    projwT = proj_w.T.reshape(2, 128, DIM).transpose(1, 0, 2)  # [128, 2, 256]
    pbias = patch_b.reshape(2, 128).T.copy()                   # [128, 2]
    obias = (proj_w @ v_b + proj_b).reshape(2, 128).T.copy()   # [128, 2]

    shared = {
        "pwT": pwT.astype(bf), "qkwT": qkwT.astype(bf),
        "wvT": wvT4.astype(bf), "projwT": projwT.astype(bf),
        "pbias": pbias.astype(np.float32), "obias": obias.astype(np.float32),
        "ident": np.eye(NCH, dtype=bf),
    }

    in_maps = []
    for b in range(B):
        # patches: [768, 196] part order (ci, ki, kj) -> [128, 6, 196]
        p4 = x[b].reshape(CIN, Hp, P, Wp, P).transpose(0, 2, 4, 1, 3)
        patches = p4.reshape(CIN * P * P, N).reshape(6, 128, N)
        patches = patches.transpose(1, 0, 2).astype(bf)
        xpad = np.zeros((CIN, H + 2, W + 2), np.float32)
        xpad[:, 1:-1, 1:-1] = x[b]
        for s in range(2):
            cols = np.empty((CIN, 3, 3, Hp, Wp, KI, P), np.float32)
            for dy in range(3):
                for dx in range(3):
                    view = xpad[:, dy:dy + H, dx:dx + W]
                    v4 = view.reshape(CIN, Hp, P, Wp, P)[:, :, 8 * s:8 * s + 8]
                    cols[:, dy, dx] = v4.transpose(0, 1, 3, 2, 4)
            # [27, (196 m, 128 off)] -> (o-major, m-minor), 2 groups of
            # 64 o at 32-aligned partition bases
            xc = cols.reshape(27, N, OFF).transpose(0, 2, 1)   # [27, off, m]
            xcol = np.zeros((64, 64 * N), np.float32)
            xcol[0:27] = xc[:, :64].reshape(27, 64 * N)
            xcol[32:59] = xc[:, 64:].reshape(27, 64 * N)
            xcol = xcol.astype(bf)
            in_maps.append(dict(shared, xcol=xcol, patches=patches))
    return in_maps


def kernel(**inputs):
    if "nc" not in _CACHE:
        _CACHE["nc"] = _build()
    nc = _CACHE["nc"]
    in_maps = _host_prep(inputs)
    res = run_bass_kernel_spmd(nc, in_maps, core_ids=list(range(8)))
    out = np.zeros((B, DIM, H, W), np.float32)
    ov = out.reshape(B, DIM, Hp, P, Wp, P)
    for i, r in enumerate(res.results):
        b, s = divmod(i, 2)
        # out cols = (off, n) = (ki, kj, hp, wp)
        o = np.asarray(r["out"], dtype=np.float32)
        o = o.reshape(DIM, KI, P, Hp, Wp)
        ov[b, :, :, 8 * s:8 * s + 8, :, :] = o.transpose(0, 3, 1, 4, 2)
    return out
